# revision 23
# baseline (speedup 1.0000x reference)
"""Trainium2 Bass kernel for nn_CSABlock (dual spatial-attention gating).

Reference computation:
    sa_x  = sigmoid(conv3d(concat[max_c(x), mean_c(x)], w, k=7, pad=3))
    fix_out  = move * sa_fix + fix
    move_out = fix * sa_move + move

Sharding: 8 cores = (batch 2) x (D quarters of 20 planes). Each core gets a
zero-padded bf16 input slab of 28 D-planes (4 pad each side; conv halo needs
3) per tensor and produces 20 output planes in bf16. Host casts/shards/pads/
gathers (rel-err budget 2e-2 >> bf16 quantization ~6e-3).

Per-core pipeline (single HBM read of each input byte):
  - Stream D in chunks of G=4 planes into a combined tile L[(d4,hg32),
    t2, c16, hp3*w96] bf16 (576B contiguous lines, one DMA per tensor).
  - Channel max/sum via tensor-tensor trees over both tensors at once
    (max on DVE, sum split DVE/GpSimd); final level casts to fp8e4.
  - Pooled stats round-trip through a DRAM bounce tile to reach the conv
    layout P[hin_pad128, stat2, dp28, wp102] fp8 (2 big DMAs per chunk
    instead of a 12-DMA partition-scatter).
  - Conv: fp8 DoubleRow matmuls; the (stat, h_in) pair packs into the
    256-deep contraction, so taps = (kd, kw) = 49 per 4-plane group with
    kh folded into the banded lhsT [hin, tap, stat, hout] (mean's 1/16
    folded into the avg-channel weights).
  - Sigmoid on ScalarE (PSUM -> SBUF bf16), gate round-trips through DRAM
    to the data layout, broadcast over c.
  - Gating: full-C bf16 mul+add on DVE (2x mode), one store per (group,
    tensor) back to DRAM.
"""

import sys

import numpy as np

for _p in ("/opt/trn_rl_repo",):
    if _p not in sys.path:
        sys.path.insert(0, _p)

import ml_dtypes  # noqa: E402

B, C, D, H, W = 2, 16, 80, 96, 96
KK = 7
DSLAB = 28          # padded per-core D planes (4 + 20 + 4)
OUTD = 20           # output planes per core
G = 4               # D planes per chunk / conv group / elementwise group
NCHUNK = DSLAB // G  # 7
NG = OUTD // G       # 5 elementwise groups
GC = 5               # conv-group D planes (free 5*102-6 = 504 <= 512)
NCG = OUTD // GC     # 4 conv groups
HG, HPW = 32, 3      # h = hg*3 + hp
WPAD = 102
NCORES = 8

CONV_FP8 = True     # fp8e4 DoubleRow conv (49 taps) vs bf16 (98 taps)

_prog_cache: dict = {}

_bf16 = ml_dtypes.bfloat16
_f8 = ml_dtypes.float8_e4m3


def _build_banded_fp8(w: np.ndarray, mean_scale: float) -> np.ndarray:
    """w: [1,2,7,7,7] f32 -> lhsT [hin_pad 128, tap 49, stat 2, hout 96] fp8.

    out[h,*] += lhsT[h+kh, (kd,kw), s, h] * P[h+kh, s, o+1+kd, w+kw]
    """
    A = np.zeros((128, KK * KK, 2, 96), np.float32)
    hh = np.arange(96)
    for s in range(2):
        scale = 1.0 if s == 0 else mean_scale
        for kd in range(KK):
            for kw in range(KK):
                tap = kd * KK + kw
                for kh in range(KK):
                    A[hh + kh, tap, s, hh] = w[0, s, kd, kh, kw] * scale
    return A.astype(_f8)


def _build_banded_bf16(w: np.ndarray, mean_scale: float) -> np.ndarray:
    """w: [1,2,7,7,7] f32 -> lhsT [hin_pad 128, tap 98, hout 96] bf16."""
    A = np.zeros((128, 2 * KK * KK, 96), np.float32)
    hh = np.arange(96)
    for s in range(2):
        scale = 1.0 if s == 0 else mean_scale
        for kd in range(KK):
            for kw in range(KK):
                tap = (s * KK + kd) * KK + kw
                for kh in range(KK):
                    A[hh + kh, tap, hh] = w[0, s, kd, kh, kw] * scale
    return A.astype(_bf16)


def _build_program():
    import concourse.bass as bass  # noqa: F401
    import concourse.bacc as bacc
    import concourse.tile as tile
    from concourse import mybir
    from contextlib import ExitStack

    f32 = mybir.dt.float32
    bf16 = mybir.dt.bfloat16
    f16 = mybir.dt.float16
    f8 = mybir.dt.float8e4
    pdt = f8 if CONV_FP8 else bf16
    TAPS = KK * KK if CONV_FP8 else 2 * KK * KK

    nc = bacc.Bacc("TRN2")
    fxs = nc.dram_tensor("fxs", [C, DSLAB, H, W], bf16, kind="ExternalInput")
    mvs = nc.dram_tensor("mvs", [C, DSLAB, H, W], bf16, kind="ExternalInput")
    if CONV_FP8:
        wgf = nc.dram_tensor("wgf", [128, TAPS, 2, 96], f8, kind="ExternalInput")
        wgm = nc.dram_tensor("wgm", [128, TAPS, 2, 96], f8, kind="ExternalInput")
    else:
        wgf = nc.dram_tensor("wgf", [128, TAPS, 96], bf16, kind="ExternalInput")
        wgm = nc.dram_tensor("wgm", [128, TAPS, 96], bf16, kind="ExternalInput")
    fo = nc.dram_tensor("fo", [C, OUTD, H, W], bf16, kind="ExternalOutput")
    mo = nc.dram_tensor("mo", [C, OUTD, H, W], bf16, kind="ExternalOutput")

    with tile.TileContext(nc) as tc, ExitStack() as ctx:
        singles = ctx.enter_context(tc.tile_pool(name="singles", bufs=1))
        # chunks 1-5 feed elementwise late -> each needs its own slot;
        # chunks 0/6 are halo-only (disjoint lifetimes, 1 shared slot)
        lp = ctx.enter_context(tc.tile_pool(name="lp", bufs=5))
        lphalo = ctx.enter_context(tc.tile_pool(name="lph", bufs=1))
        trpool = ctx.enter_context(tc.tile_pool(name="tr", bufs=1))
        pspool = ctx.enter_context(tc.tile_pool(name="ps", bufs=2))
        tpool = ctx.enter_context(tc.tile_pool(name="tmp", bufs=2))
        gpool = ctx.enter_context(tc.tile_pool(name="gate", bufs=2))
        gtpool = ctx.enter_context(tc.tile_pool(name="gateT", bufs=3))
        psum = ctx.enter_context(tc.tile_pool(name="psum", bufs=4, space="PSUM"))
        dram = ctx.enter_context(tc.tile_pool(name="dram", bufs=1, space="DRAM"))

        WGF = singles.tile(list(wgf.shape), pdt)
        WGM = singles.tile(list(wgm.shape), pdt)
        nc.scalar.dma_start(out=WGF[:], in_=wgf[:])
        nc.scalar.dma_start(out=WGM[:], in_=wgm[:])

        # Persistent pooled tensors [hin_pad, stat, dp*wp] ((d,w) flattened so
        # conv rhs slices are single contiguous runs; the 6 pad columns per
        # plane become ignored output columns)
        PF = singles.tile([128, 2, DSLAB * WPAD], pdt)
        PM = singles.tile([128, 2, DSLAB * WPAD], pdt)
        nc.gpsimd.memset(PF[:], 0.0)
        nc.gpsimd.memset(PM[:], 0.0)

        # DRAM bounce tiles
        pooled_d = [
            dram.tile([2, 2, G, H, W], pdt, name=f"pooled_d{i}")
            for i in range(NCHUNK)
        ]
        gates_d = {
            (t, g): dram.tile([GC, H, W], bf16, name=f"gates_d{t}_{g}")
            for t in range(2) for g in range(NCG)
        }

        ltiles: dict = {}

        def load_and_pool(ic: int):
            i0 = G * ic
            # Combined tile: partition (d4, hg32); free (t2, c16, hp3*w96)
            pool_ = lp if 1 <= ic <= 5 else lphalo
            L = pool_.tile([128, 2, C, HPW * W], bf16, tag="L")
            # slab plane 27 is never read downstream: skip its load (P's
            # memset supplies zeros; engine APs need base partition 0, so
            # plane 0 keeps its host-zeroed load)
            dlo = 0
            dhi = 3 if ic == NCHUNK - 1 else G
            PL = slice(32 * dlo, 32 * dhi)
            for t, dram_in in ((0, fxs), (1, mvs)):
                src = dram_in[:, i0 + dlo:i0 + dhi, :, :].rearrange(
                    "c d (hg hp) w -> (d hg) c (hp w)", hg=HG, hp=HPW
                )
                nc.sync.dma_start(out=L[PL, t], in_=src)
            ltiles[ic] = L

            # channel-reduction trees over both tensors at once (fp16: exact
            # for bf16 inputs, keeps DVE 2x mode, avoids bf16 sum noise)
            TR = trpool.tile([128, 2, 2, C // 2, HPW * W], f16, tag="TR")
            PS = pspool.tile([128, 2, 2, HPW * W], pdt, tag="PS")
            TRmax, TRsum = TR[PL, :, 0], TR[PL, :, 1]
            Lv = L[PL]
            # Tree ops at high priority: in the in-order DVE stream they
            # must never queue behind elementwise ops (they gate the convs).
            with tc.high_priority(offset=4000):
                # level 1: 16 -> 8
                nc.vector.tensor_max(TRmax[:, :, :, :], Lv[:, :, 0:8, :], Lv[:, :, 8:16, :])
                nc.vector.tensor_add(TRsum[:, :, :, :], Lv[:, :, 0:8, :], Lv[:, :, 8:16, :])
                # levels 2-4 all on DVE: GpSimd tensor ops hold the shared SBUF
                # port for multi-us and stall every concurrent DVE perf-mode op
                nc.vector.tensor_max(TRmax[:, :, 0:4], TRmax[:, :, 0:4], TRmax[:, :, 4:8])
                nc.vector.tensor_add(TRsum[:, :, 0:4], TRsum[:, :, 0:4], TRsum[:, :, 4:8])
                nc.vector.tensor_max(TRmax[:, :, 0:2], TRmax[:, :, 0:2], TRmax[:, :, 2:4])
                nc.vector.tensor_add(TRsum[:, :, 0:2], TRsum[:, :, 0:2], TRsum[:, :, 2:4])
                nc.vector.tensor_max(PS[PL, :, 0], TRmax[:, :, 0], TRmax[:, :, 1])
                nc.vector.tensor_add(TRsum[:, :, 0], TRsum[:, :, 0], TRsum[:, :, 1])
            # mean = sum/16 applied here on ScalarE: folding 1/16 into fp8
            # weights would push them into e4m3 subnormals
            nc.scalar.mul(PS[PL, :, 1], TRsum[:, :, 0], 1.0 / C)

            # bounce: PS [(d hg), t, s, (hp w)] -> DRAM [t, s, d, h, w]
            nc.scalar.dma_start(
                out=pooled_d[ic][:, :, dlo:dhi].rearrange(
                    "t s d (hg hp) w -> (d hg) t s (hp w)", hg=HG, hp=HPW
                ),
                in_=PS[PL],
            )
            # reload into conv layout per tensor/stat (3-dim DMA APs).
            # On the gpsimd (SWDGE) queue: these issues wait on the bounce
            # write, and on the in-order sync queue they would stall every
            # later input load behind them.
            for t, P in ((0, PF), (1, PM)):
                Pv = P[3:99, :, :].rearrange("p s (d w) -> p s d w", w=WPAD)
                for s in range(2):
                    nc.gpsimd.dma_start(
                        out=Pv[:, s, i0 + dlo:i0 + dhi, 3:3 + W],
                        in_=pooled_d[ic][t, s, dlo:dhi].rearrange("d h w -> h d w"),
                    )

        NFREE = GC * WPAD - (WPAD - W)  # 504: contiguous (d,w) run per tap

        def conv_group(t: int, g: int):
            P = (PF, PM)[t]
            WG = (WGF, WGM)[t]
            o0 = GC * g
            acc = psum.tile([96, NFREE], f32, tag="acc")
            if CONV_FP8:
                for kd in range(KK):
                    for kw in range(KK):
                        tap = kd * KK + kw
                        off = (o0 + 1 + kd) * WPAD + kw
                        nc.tensor.matmul(
                            acc[:],
                            WG[:, tap],
                            P[:, :, off:off + NFREE],
                            start=(tap == 0),
                            stop=(tap == TAPS - 1),
                            perf_mode=mybir.MatmulPerfMode.DoubleRow,
                        )
            else:
                for s in range(2):
                    for kd in range(KK):
                        for kw in range(KK):
                            tap = (s * KK + kd) * KK + kw
                            off = (o0 + 1 + kd) * WPAD + kw
                            nc.tensor.matmul(
                                acc[:],
                                WG[:, tap],
                                P[:, s, off:off + NFREE],
                                start=(tap == 0),
                                stop=(tap == TAPS - 1),
                            )
            gate = gpool.tile([96, GC * WPAD], bf16, tag="gate")
            nc.scalar.activation(
                out=gate[:, 0:NFREE], in_=acc[:],
                func=mybir.ActivationFunctionType.Sigmoid,
            )
            gv = gate[:].rearrange("p (d w) -> p d w", w=WPAD)
            nc.scalar.dma_start(
                out=gates_d[(t, g)][:].rearrange("d h w -> h d w"),
                in_=gv[:, :, 0:W],
            )

        def elementwise(g: int):
            L = ltiles[g + 1]
            Ts = []
            p0 = G * g
            for t in range(2):
                gateT = gtpool.tile([128, HPW * W], bf16, tag="gT")
                # gather 4 gate planes from the (possibly two) 5-plane
                # conv-group tiles they live in
                # scalar queue: this issue waits on the gate-write, and on
                # the in-order sync queue it would stall later input loads
                p = p0
                while p < p0 + G:
                    cg, lo = p // GC, p % GC
                    hi = min(GC, lo + (p0 + G - p))
                    nc.scalar.dma_start(
                        out=gateT[32 * (p - p0):32 * (p - p0 + hi - lo)],
                        in_=gates_d[(t, cg)][lo:hi].rearrange(
                            "d (hg hp) w -> (d hg) (hp w)", hg=HG, hp=HPW
                        ),
                    )
                    p += hi - lo
                gb = gateT[:].unsqueeze(1).broadcast_to((128, C, HPW * W))
                T = tpool.tile([128, C, HPW * W], bf16, tag="T")
                # t=0: fix_out = move*gf + fix ; t=1: move_out = fix*gm + move
                nc.vector.tensor_mul(T[:], L[:, 1 - t], gb)
                nc.vector.tensor_add(T[:], T[:], L[:, t])
                Ts.append(T)
            for t, dram_out in ((0, fo), (1, mo)):
                dst = dram_out[:, p0:p0 + G, :, :].rearrange(
                    "c d (hg hp) w -> (d hg) c (hp w)", hg=HG, hp=HPW
                )
                nc.scalar.dma_start(out=dst, in_=Ts[t][:])

        # software pipeline. conv group g (5 planes) needs pooled chunks
        # <= ceil((5g+11)/4)-1; elementwise g (4 planes) needs raw chunk
        # g+1 and the gates covering planes 4g..4g+3.
        load_and_pool(0)
        load_and_pool(1)
        load_and_pool(2)
        conv_group(0, 0)
        conv_group(1, 0)
        load_and_pool(3)
        load_and_pool(4)
        conv_group(0, 1)
        conv_group(1, 1)
        load_and_pool(5)
        load_and_pool(6)
        elementwise(0)
        elementwise(1)
        conv_group(0, 2)
        conv_group(1, 2)
        elementwise(2)
        conv_group(0, 3)
        conv_group(1, 3)
        elementwise(3)
        elementwise(4)

    nc.compile()
    return nc


def _get_program():
    if "nc" not in _prog_cache:
        _prog_cache["nc"] = _build_program()
    return _prog_cache["nc"]


def _shard(fix, move, Af, Am):
    in_maps = []
    for core in range(NCORES):
        b, dq = core // 4, core % 4
        lo = 20 * dq - 4
        s0, s1 = max(lo, 0), min(lo + DSLAB, D)
        slab_f = np.zeros((C, DSLAB, H, W), _bf16)
        slab_m = np.zeros((C, DSLAB, H, W), _bf16)
        slab_f[:, s0 - lo:s1 - lo] = fix[b, :, s0:s1].astype(_bf16)
        slab_m[:, s0 - lo:s1 - lo] = move[b, :, s0:s1].astype(_bf16)
        in_maps.append({"fxs": slab_f, "mvs": slab_m, "wgf": Af, "wgm": Am})
    return in_maps


def _build_weights(w_f2m, w_m2f):
    build = _build_banded_fp8 if CONV_FP8 else _build_banded_bf16
    Af = build(np.asarray(w_f2m, dtype=np.float32), 1.0)
    Am = build(np.asarray(w_m2f, dtype=np.float32), 1.0)
    return Af, Am


def kernel(fix, move, w_f2m, w_m2f, __trace=False):
    fix = np.ascontiguousarray(np.asarray(fix), dtype=np.float32)
    move = np.ascontiguousarray(np.asarray(move), dtype=np.float32)
    Af, Am = _build_weights(w_f2m, w_m2f)

    nc = _get_program()
    in_maps = _shard(fix, move, Af, Am)

    from concourse.bass_utils import run_bass_kernel_spmd

    res = run_bass_kernel_spmd(
        nc, in_maps, core_ids=list(range(NCORES)), trace=__trace
    )
    _prog_cache["last_results"] = res

    fix_out = np.empty((B, C, D, H, W), np.float32)
    move_out = np.empty((B, C, D, H, W), np.float32)
    for core in range(NCORES):
        b, dq = core // 4, core % 4
        fix_out[b, :, 20 * dq:20 * dq + 20] = res.results[core]["fo"].astype(np.float32)
        move_out[b, :, 20 * dq:20 * dq + 20] = res.results[core]["mo"].astype(np.float32)
    return fix_out, move_out


# revision 24
# speedup vs baseline: 1.0888x; 1.0888x over previous
"""Trainium2 Bass kernel for nn_CSABlock (dual spatial-attention gating).

v3 configuration (HW-measured 194.4 us): fp8 DoubleRow conv, bf16 I/O,
fp16 trees all on DVE, DRAM bounce staging, G=4 conv groups aligned with
elementwise groups.
"""

import sys

import numpy as np

for _p in ("/opt/trn_rl_repo",):
    if _p not in sys.path:
        sys.path.insert(0, _p)

import ml_dtypes  # noqa: E402

B, C, D, H, W = 2, 16, 80, 96, 96
KK = 7
DSLAB = 28
OUTD = 20
G = 4
NCHUNK = DSLAB // G  # 7
NG = OUTD // G       # 5
HG, HPW = 32, 3
WPAD = 102
NCORES = 8

CONV_FP8 = True

_prog_cache: dict = {}

_bf16 = ml_dtypes.bfloat16
_f8 = ml_dtypes.float8_e4m3


def _build_banded_fp8(w: np.ndarray, mean_scale: float) -> np.ndarray:
    A = np.zeros((128, KK * KK, 2, 96), np.float32)
    hh = np.arange(96)
    for s in range(2):
        scale = 1.0 if s == 0 else mean_scale
        for kd in range(KK):
            for kw in range(KK):
                tap = kd * KK + kw
                for kh in range(KK):
                    A[hh + kh, tap, s, hh] = w[0, s, kd, kh, kw] * scale
    return A.astype(_f8)


def _build_banded_bf16(w: np.ndarray, mean_scale: float) -> np.ndarray:
    A = np.zeros((128, 2 * KK * KK, 96), np.float32)
    hh = np.arange(96)
    for s in range(2):
        scale = 1.0 if s == 0 else mean_scale
        for kd in range(KK):
            for kw in range(KK):
                tap = (s * KK + kd) * KK + kw
                for kh in range(KK):
                    A[hh + kh, tap, hh] = w[0, s, kd, kh, kw] * scale
    return A.astype(_bf16)


def _build_program():
    import concourse.bass as bass  # noqa: F401
    import concourse.bacc as bacc
    import concourse.tile as tile
    from concourse import mybir
    from contextlib import ExitStack

    f32 = mybir.dt.float32
    bf16 = mybir.dt.bfloat16
    f16 = mybir.dt.float16
    f8 = mybir.dt.float8e4
    pdt = f8 if CONV_FP8 else bf16
    TAPS = KK * KK if CONV_FP8 else 2 * KK * KK

    nc = bacc.Bacc("TRN2")
    fxs = nc.dram_tensor("fxs", [C, DSLAB, H, W], bf16, kind="ExternalInput")
    mvs = nc.dram_tensor("mvs", [C, DSLAB, H, W], bf16, kind="ExternalInput")
    if CONV_FP8:
        wgf = nc.dram_tensor("wgf", [128, TAPS, 2, 96], f8, kind="ExternalInput")
        wgm = nc.dram_tensor("wgm", [128, TAPS, 2, 96], f8, kind="ExternalInput")
    else:
        wgf = nc.dram_tensor("wgf", [128, TAPS, 96], bf16, kind="ExternalInput")
        wgm = nc.dram_tensor("wgm", [128, TAPS, 96], bf16, kind="ExternalInput")
    fo = nc.dram_tensor("fo", [C, OUTD, H, W], bf16, kind="ExternalOutput")
    mo = nc.dram_tensor("mo", [C, OUTD, H, W], bf16, kind="ExternalOutput")

    with tile.TileContext(nc) as tc, ExitStack() as ctx:
        singles = ctx.enter_context(tc.tile_pool(name="singles", bufs=1))
        lp = ctx.enter_context(tc.tile_pool(name="lp", bufs=4))
        trpool = ctx.enter_context(tc.tile_pool(name="tr", bufs=2))
        pspool = ctx.enter_context(tc.tile_pool(name="ps", bufs=2))
        tpool = ctx.enter_context(tc.tile_pool(name="tmp", bufs=3))
        gpool = ctx.enter_context(tc.tile_pool(name="gate", bufs=2))
        gtpool = ctx.enter_context(tc.tile_pool(name="gateT", bufs=3))
        psum = ctx.enter_context(tc.tile_pool(name="psum", bufs=4, space="PSUM"))
        dram = ctx.enter_context(tc.tile_pool(name="dram", bufs=1, space="DRAM"))

        WGF = singles.tile(list(wgf.shape), pdt)
        WGM = singles.tile(list(wgm.shape), pdt)
        nc.scalar.dma_start(out=WGF[:], in_=wgf[:])
        nc.scalar.dma_start(out=WGM[:], in_=wgm[:])

        PF = singles.tile([128, 2, DSLAB * WPAD], pdt)
        PM = singles.tile([128, 2, DSLAB * WPAD], pdt)
        nc.gpsimd.memset(PF[:], 0.0)
        nc.gpsimd.memset(PM[:], 0.0)

        pooled_d = [
            dram.tile([2, 2, G, H, W], pdt, name=f"pooled_d{i}")
            for i in range(NCHUNK)
        ]
        gates_d = {
            (t, g): dram.tile([G, H, W], bf16, name=f"gates_d{t}_{g}")
            for t in range(2) for g in range(NG)
        }

        ltiles: dict = {}

        def load_and_pool(ic: int):
            i0 = G * ic
            L = lp.tile([128, 2, C, HPW * W], bf16, tag="L")
            for t, dram_in in ((0, fxs), (1, mvs)):
                src = dram_in[:, i0:i0 + G, :, :].rearrange(
                    "c d (hg hp) w -> (d hg) c (hp w)", hg=HG, hp=HPW
                )
                nc.sync.dma_start(out=L[:, t], in_=src)
            ltiles[ic] = L

            TR = trpool.tile([128, 2, 2, C // 2, HPW * W], f16, tag="TR")
            PS = pspool.tile([128, 2, 2, HPW * W], pdt, tag="PS")
            TRmax, TRsum = TR[:, :, 0], TR[:, :, 1]
            nc.vector.tensor_max(TRmax[:, :, :, :], L[:, :, 0:8, :], L[:, :, 8:16, :])
            nc.vector.tensor_add(TRsum[:, :, :, :], L[:, :, 0:8, :], L[:, :, 8:16, :])
            nc.vector.tensor_max(TRmax[:, :, 0:4], TRmax[:, :, 0:4], TRmax[:, :, 4:8])
            nc.vector.tensor_add(TRsum[:, :, 0:4], TRsum[:, :, 0:4], TRsum[:, :, 4:8])
            nc.vector.tensor_max(TRmax[:, :, 0:2], TRmax[:, :, 0:2], TRmax[:, :, 2:4])
            nc.vector.tensor_add(TRsum[:, :, 0:2], TRsum[:, :, 0:2], TRsum[:, :, 2:4])
            nc.vector.tensor_max(PS[:, :, 0], TRmax[:, :, 0], TRmax[:, :, 1])
            nc.vector.tensor_add(TRsum[:, :, 0], TRsum[:, :, 0], TRsum[:, :, 1])
            nc.scalar.mul(PS[:, :, 1], TRsum[:, :, 0], 1.0 / C)

            nc.scalar.dma_start(
                out=pooled_d[ic][:].rearrange(
                    "t s d (hg hp) w -> (d hg) t s (hp w)", hg=HG, hp=HPW
                ),
                in_=PS[:],
            )
            for t, P in ((0, PF), (1, PM)):
                Pv = P[3:99, :, :].rearrange("p s (d w) -> p s d w", w=WPAD)
                for s in range(2):
                    nc.sync.dma_start(
                        out=Pv[:, s, i0:i0 + G, 3:3 + W],
                        in_=pooled_d[ic][t, s].rearrange("d h w -> h d w"),
                    )

        NFREE = G * WPAD - (WPAD - W)  # 402

        def conv_group(t: int, g: int):
            P = (PF, PM)[t]
            WG = (WGF, WGM)[t]
            o0 = G * g
            acc = psum.tile([96, NFREE], f32, tag="acc")
            if CONV_FP8:
                for kd in range(KK):
                    for kw in range(KK):
                        tap = kd * KK + kw
                        off = (o0 + 1 + kd) * WPAD + kw
                        nc.tensor.matmul(
                            acc[:],
                            WG[:, tap],
                            P[:, :, off:off + NFREE],
                            start=(tap == 0),
                            stop=(tap == TAPS - 1),
                            perf_mode=mybir.MatmulPerfMode.DoubleRow,
                        )
            else:
                for s in range(2):
                    for kd in range(KK):
                        for kw in range(KK):
                            tap = (s * KK + kd) * KK + kw
                            off = (o0 + 1 + kd) * WPAD + kw
                            nc.tensor.matmul(
                                acc[:],
                                WG[:, tap],
                                P[:, s, off:off + NFREE],
                                start=(tap == 0),
                                stop=(tap == TAPS - 1),
                            )
            gate = gpool.tile([96, G * WPAD], bf16, tag="gate")
            nc.scalar.activation(
                out=gate[:, 0:NFREE], in_=acc[:],
                func=mybir.ActivationFunctionType.Sigmoid,
            )
            gv = gate[:].rearrange("p (d w) -> p d w", w=WPAD)
            nc.scalar.dma_start(
                out=gates_d[(t, g)][:].rearrange("d h w -> h d w"),
                in_=gv[:, :, 0:W],
            )

        def elementwise(g: int):
            L = ltiles[g + 1]
            Ts = []
            for t in range(2):
                gateT = gtpool.tile([128, HPW * W], bf16, tag="gT")
                nc.sync.dma_start(
                    out=gateT[:],
                    in_=gates_d[(t, g)][:].rearrange(
                        "d (hg hp) w -> (d hg) (hp w)", hg=HG, hp=HPW
                    ),
                )
                gb = gateT[:].unsqueeze(1).broadcast_to((128, C, HPW * W))
                T = tpool.tile([128, C, HPW * W], bf16, tag="T")
                nc.vector.tensor_mul(T[:], L[:, 1 - t], gb)
                nc.vector.tensor_add(T[:], T[:], L[:, t])
                Ts.append(T)
            for t, dram_out in ((0, fo), (1, mo)):
                dst = dram_out[:, G * g:G * g + G, :, :].rearrange(
                    "c d (hg hp) w -> (d hg) c (hp w)", hg=HG, hp=HPW
                )
                nc.scalar.dma_start(out=dst, in_=Ts[t][:])

        load_and_pool(0)
        load_and_pool(1)
        load_and_pool(2)
        for g in range(NG):
            if g + 3 < NCHUNK:
                load_and_pool(g + 3)
            conv_group(0, g)
            conv_group(1, g)
            elementwise(g)

    nc.compile()
    return nc


def _get_program():
    if "nc" not in _prog_cache:
        _prog_cache["nc"] = _build_program()
    return _prog_cache["nc"]


def _shard(fix, move, Af, Am):
    in_maps = []
    for core in range(NCORES):
        b, dq = core // 4, core % 4
        lo = 20 * dq - 4
        s0, s1 = max(lo, 0), min(lo + DSLAB, D)
        slab_f = np.zeros((C, DSLAB, H, W), _bf16)
        slab_m = np.zeros((C, DSLAB, H, W), _bf16)
        slab_f[:, s0 - lo:s1 - lo] = fix[b, :, s0:s1].astype(_bf16)
        slab_m[:, s0 - lo:s1 - lo] = move[b, :, s0:s1].astype(_bf16)
        in_maps.append({"fxs": slab_f, "mvs": slab_m, "wgf": Af, "wgm": Am})
    return in_maps


def _build_weights(w_f2m, w_m2f):
    build = _build_banded_fp8 if CONV_FP8 else _build_banded_bf16
    Af = build(np.asarray(w_f2m, dtype=np.float32), 1.0)
    Am = build(np.asarray(w_m2f, dtype=np.float32), 1.0)
    return Af, Am


def kernel(fix, move, w_f2m, w_m2f, __trace=False):
    fix = np.ascontiguousarray(np.asarray(fix), dtype=np.float32)
    move = np.ascontiguousarray(np.asarray(move), dtype=np.float32)
    Af, Am = _build_weights(w_f2m, w_m2f)

    nc = _get_program()
    in_maps = _shard(fix, move, Af, Am)

    from concourse.bass_utils import run_bass_kernel_spmd

    res = run_bass_kernel_spmd(
        nc, in_maps, core_ids=list(range(NCORES)), trace=__trace
    )
    _prog_cache["last_results"] = res

    fix_out = np.empty((B, C, D, H, W), np.float32)
    move_out = np.empty((B, C, D, H, W), np.float32)
    for core in range(NCORES):
        b, dq = core // 4, core % 4
        fix_out[b, :, 20 * dq:20 * dq + 20] = res.results[core]["fo"].astype(np.float32)
        move_out[b, :, 20 * dq:20 * dq + 20] = res.results[core]["mo"].astype(np.float32)
    return fix_out, move_out


# revision 25
# speedup vs baseline: 1.1666x; 1.0714x over previous
"""Trainium2 Bass kernel for nn_CSABlock (dual spatial-attention gating).

Reference computation:
    sa_x  = sigmoid(conv3d(concat[max_c(x), mean_c(x)], w, k=7, pad=3))
    fix_out  = move * sa_fix + fix
    move_out = fix * sa_move + move

Sharding: 8 cores = (batch 2) x (D quarters of 20 planes); bf16 28-plane
slabs with 4-plane halo/pad. fp8 DoubleRow conv (stats packed into the
256-deep contraction), fp16 channel trees on DVE, DRAM bounce staging.

Queue discipline (all DMA-issue queues are in-order; a dependency-waiting
issue stalls everything behind it on that queue):
  sync   - input loads first (never wait), then late gateT reloads
  scalar - PS scale, pooled bounce writes, sigmoids, gate writes, stores
           (emission interleaved to match readiness order)
  gpsimd - P reloads (wait on bounce writes; engine otherwise idle)
"""

import sys

import numpy as np

for _p in ("/opt/trn_rl_repo",):
    if _p not in sys.path:
        sys.path.insert(0, _p)

import ml_dtypes  # noqa: E402

B, C, D, H, W = 2, 16, 80, 96, 96
KK = 7
DSLAB = 28          # padded per-core D planes (4 + 20 + 4)
OUTD = 20           # output planes per core
G = 4               # D planes per chunk / elementwise group
NCHUNK = DSLAB // G  # 7
NG = OUTD // G       # 5 elementwise groups
GC = 5               # conv-group D planes (free 5*102-6 = 504 <= 512)
NCG = OUTD // GC     # 4 conv groups
HG, HPW = 32, 3      # h = hg*3 + hp
WPAD = 102
NCORES = 8

CONV_FP8 = True

_prog_cache: dict = {}

_bf16 = ml_dtypes.bfloat16
_f8 = ml_dtypes.float8_e4m3


def _build_banded_fp8(w: np.ndarray, mean_scale: float) -> np.ndarray:
    """w: [1,2,7,7,7] f32 -> lhsT [hin_pad 128, tap 49, stat 2, hout 96] fp8."""
    A = np.zeros((128, KK * KK, 2, 96), np.float32)
    hh = np.arange(96)
    for s in range(2):
        scale = 1.0 if s == 0 else mean_scale
        for kd in range(KK):
            for kw in range(KK):
                tap = kd * KK + kw
                for kh in range(KK):
                    A[hh + kh, tap, s, hh] = w[0, s, kd, kh, kw] * scale
    return A.astype(_f8)


def _build_banded_bf16(w: np.ndarray, mean_scale: float) -> np.ndarray:
    """w: [1,2,7,7,7] f32 -> lhsT [hin_pad 128, tap 98, hout 96] bf16."""
    A = np.zeros((128, 2 * KK * KK, 96), np.float32)
    hh = np.arange(96)
    for s in range(2):
        scale = 1.0 if s == 0 else mean_scale
        for kd in range(KK):
            for kw in range(KK):
                tap = (s * KK + kd) * KK + kw
                for kh in range(KK):
                    A[hh + kh, tap, hh] = w[0, s, kd, kh, kw] * scale
    return A.astype(_bf16)


def _build_program():
    import concourse.bass as bass  # noqa: F401
    import concourse.bacc as bacc
    import concourse.tile as tile
    from concourse import mybir
    from contextlib import ExitStack

    f32 = mybir.dt.float32
    bf16 = mybir.dt.bfloat16
    f16 = mybir.dt.float16
    f8 = mybir.dt.float8e4
    pdt = f8 if CONV_FP8 else bf16
    TAPS = KK * KK if CONV_FP8 else 2 * KK * KK

    nc = bacc.Bacc("TRN2")
    fxs = nc.dram_tensor("fxs", [C, DSLAB, H, W], bf16, kind="ExternalInput")
    mvs = nc.dram_tensor("mvs", [C, DSLAB, H, W], bf16, kind="ExternalInput")
    if CONV_FP8:
        wgf = nc.dram_tensor("wgf", [128, TAPS, 2, 96], f8, kind="ExternalInput")
        wgm = nc.dram_tensor("wgm", [128, TAPS, 2, 96], f8, kind="ExternalInput")
    else:
        wgf = nc.dram_tensor("wgf", [128, TAPS, 96], bf16, kind="ExternalInput")
        wgm = nc.dram_tensor("wgm", [128, TAPS, 96], bf16, kind="ExternalInput")
    fo = nc.dram_tensor("fo", [C, OUTD, H, W], bf16, kind="ExternalOutput")
    mo = nc.dram_tensor("mo", [C, OUTD, H, W], bf16, kind="ExternalOutput")

    with tile.TileContext(nc) as tc, ExitStack() as ctx:
        singles = ctx.enter_context(tc.tile_pool(name="singles", bufs=1))
        # chunks 1-5 feed elementwise late -> own slots so loads never wait;
        # chunks 0/6 are halo-only with disjoint lifetimes (1 shared slot)
        lp = ctx.enter_context(tc.tile_pool(name="lp", bufs=5))
        lphalo = ctx.enter_context(tc.tile_pool(name="lph", bufs=1))
        trpool = ctx.enter_context(tc.tile_pool(name="tr", bufs=1))
        pspool = ctx.enter_context(tc.tile_pool(name="ps", bufs=2))
        tpool = ctx.enter_context(tc.tile_pool(name="tmp", bufs=2))
        gpool = ctx.enter_context(tc.tile_pool(name="gate", bufs=2))
        gtpool = ctx.enter_context(tc.tile_pool(name="gateT", bufs=3))
        psum = ctx.enter_context(tc.tile_pool(name="psum", bufs=4, space="PSUM"))
        dram = ctx.enter_context(tc.tile_pool(name="dram", bufs=1, space="DRAM"))

        WGF = singles.tile(list(wgf.shape), pdt)
        WGM = singles.tile(list(wgm.shape), pdt)
        nc.scalar.dma_start(out=WGF[:], in_=wgf[:])
        nc.scalar.dma_start(out=WGM[:], in_=wgm[:])

        # Persistent pooled tensors [hin_pad, stat, dp*wp]; (d,w) flattened
        # so conv rhs slices are single contiguous runs
        PF = singles.tile([128, 2, DSLAB * WPAD], pdt)
        PM = singles.tile([128, 2, DSLAB * WPAD], pdt)
        nc.gpsimd.memset(PF[:], 0.0)
        nc.gpsimd.memset(PM[:], 0.0)

        pooled_d = [
            dram.tile([2, 2, G, H, W], pdt, name=f"pooled_d{i}")
            for i in range(NCHUNK)
        ]
        gates_d = {
            (t, g): dram.tile([GC, H, W], bf16, name=f"gates_d{t}_{g}")
            for t in range(2) for g in range(NCG)
        }

        ltiles: dict = {}

        def load_and_pool(ic: int):
            i0 = G * ic
            pool_ = lp if 1 <= ic <= 5 else lphalo
            L = pool_.tile([128, 2, C, HPW * W], bf16, tag="L")
            for t, dram_in in ((0, fxs), (1, mvs)):
                src = dram_in[:, i0:i0 + G, :, :].rearrange(
                    "c d (hg hp) w -> (d hg) c (hp w)", hg=HG, hp=HPW
                )
                nc.sync.dma_start(out=L[:, t], in_=src)
            ltiles[ic] = L

            # channel trees over both tensors at once, fp16, all on DVE
            # (GpSimd tensor ops would hold the shared SBUF port and stall
            # DVE perf-mode ops). High priority: in the in-order DVE stream
            # trees must never queue behind elementwise - they gate convs.
            TR = trpool.tile([128, 2, 2, C // 2, HPW * W], f16, tag="TR")
            PS = pspool.tile([128, 2, 2, HPW * W], pdt, tag="PS")
            TRmax, TRsum = TR[:, :, 0], TR[:, :, 1]
            with tc.high_priority(offset=4000):
                nc.vector.tensor_max(TRmax[:, :, :, :], L[:, :, 0:8, :], L[:, :, 8:16, :])
                nc.vector.tensor_add(TRsum[:, :, :, :], L[:, :, 0:8, :], L[:, :, 8:16, :])
                nc.vector.tensor_max(TRmax[:, :, 0:4], TRmax[:, :, 0:4], TRmax[:, :, 4:8])
                nc.vector.tensor_add(TRsum[:, :, 0:4], TRsum[:, :, 0:4], TRsum[:, :, 4:8])
                nc.vector.tensor_max(TRmax[:, :, 0:2], TRmax[:, :, 0:2], TRmax[:, :, 2:4])
                nc.vector.tensor_add(TRsum[:, :, 0:2], TRsum[:, :, 0:2], TRsum[:, :, 2:4])
                nc.vector.tensor_max(PS[:, :, 0], TRmax[:, :, 0], TRmax[:, :, 1])
                nc.vector.tensor_add(TRsum[:, :, 0], TRsum[:, :, 0], TRsum[:, :, 1])
            # mean = sum/16 on ScalarE (1/16 in fp8 weights would hit e4m3
            # subnormals)
            nc.scalar.mul(PS[:, :, 1], TRsum[:, :, 0], 1.0 / C)

            # bounce out: PS [(d hg), t, s, (hp w)] -> DRAM [t, s, d, h, w]
            nc.scalar.dma_start(
                out=pooled_d[ic][:].rearrange(
                    "t s d (hg hp) w -> (d hg) t s (hp w)", hg=HG, hp=HPW
                ),
                in_=PS[:],
            )
            # reload into conv layout, gpsimd queue (waits on the bounce
            # write; on sync it would stall later input loads)
            for t, P in ((0, PF), (1, PM)):
                Pv = P[3:99, :, :].rearrange("p s (d w) -> p s d w", w=WPAD)
                for s in range(2):
                    nc.gpsimd.dma_start(
                        out=Pv[:, s, i0:i0 + G, 3:3 + W],
                        in_=pooled_d[ic][t, s].rearrange("d h w -> h d w"),
                    )

        NFREE = GC * WPAD - (WPAD - W)  # 504: contiguous (d,w) run per tap

        def conv_group(t: int, g: int):
            P = (PF, PM)[t]
            WG = (WGF, WGM)[t]
            o0 = GC * g
            acc = psum.tile([96, NFREE], f32, tag="acc")
            if CONV_FP8:
                for kd in range(KK):
                    for kw in range(KK):
                        tap = kd * KK + kw
                        off = (o0 + 1 + kd) * WPAD + kw
                        nc.tensor.matmul(
                            acc[:],
                            WG[:, tap],
                            P[:, :, off:off + NFREE],
                            start=(tap == 0),
                            stop=(tap == TAPS - 1),
                            perf_mode=mybir.MatmulPerfMode.DoubleRow,
                        )
            else:
                for s in range(2):
                    for kd in range(KK):
                        for kw in range(KK):
                            tap = (s * KK + kd) * KK + kw
                            off = (o0 + 1 + kd) * WPAD + kw
                            nc.tensor.matmul(
                                acc[:],
                                WG[:, tap],
                                P[:, s, off:off + NFREE],
                                start=(tap == 0),
                                stop=(tap == TAPS - 1),
                            )
            gate = gpool.tile([96, GC * WPAD], bf16, tag="gate")
            nc.scalar.activation(
                out=gate[:, 0:NFREE], in_=acc[:],
                func=mybir.ActivationFunctionType.Sigmoid,
            )
            gv = gate[:].rearrange("p (d w) -> p d w", w=WPAD)
            nc.scalar.dma_start(
                out=gates_d[(t, g)][:].rearrange("d h w -> h d w"),
                in_=gv[:, :, 0:W],
            )

        def elementwise(g: int):
            L = ltiles[g + 1]
            Ts = []
            p0 = G * g
            for t in range(2):
                gateT = gtpool.tile([128, HPW * W], bf16, tag="gT")
                # gather 4 gate planes from the (up to two) 5-plane conv
                # group tiles; sync queue is pure loads before these, all
                # already issued
                p = p0
                while p < p0 + G:
                    cg, lo = p // GC, p % GC
                    hi = min(GC, lo + (p0 + G - p))
                    nc.sync.dma_start(
                        out=gateT[32 * (p - p0):32 * (p - p0 + hi - lo)],
                        in_=gates_d[(t, cg)][lo:hi].rearrange(
                            "d (hg hp) w -> (d hg) (hp w)", hg=HG, hp=HPW
                        ),
                    )
                    p += hi - lo
                gb = gateT[:].unsqueeze(1).broadcast_to((128, C, HPW * W))
                T = tpool.tile([128, C, HPW * W], bf16, tag="T")
                # t=0: fix_out = move*gf + fix ; t=1: move_out = fix*gm + move
                nc.vector.tensor_mul(T[:], L[:, 1 - t], gb)
                nc.vector.tensor_add(T[:], T[:], L[:, t])
                Ts.append(T)
            for t, dram_out in ((0, fo), (1, mo)):
                dst = dram_out[:, p0:p0 + G, :, :].rearrange(
                    "c d (hg hp) w -> (d hg) c (hp w)", hg=HG, hp=HPW
                )
                nc.scalar.dma_start(out=dst, in_=Ts[t][:])

        # Pipeline: all loads emitted first (sync queue = loads only, and
        # lp slots guarantee no allocation wait); conv group g needs pooled
        # chunks <= [2,4,5,6][g]; elementwise g needs gates covering planes
        # 4g..4g+3 and raw chunk g+1.
        for ic in range(NCHUNK):
            load_and_pool(ic)
        conv_group(0, 0)
        conv_group(1, 0)
        conv_group(0, 1)
        conv_group(1, 1)
        elementwise(0)
        elementwise(1)
        conv_group(0, 2)
        conv_group(1, 2)
        elementwise(2)
        conv_group(0, 3)
        conv_group(1, 3)
        elementwise(3)
        elementwise(4)

    nc.compile()
    return nc


def _get_program():
    if "nc" not in _prog_cache:
        _prog_cache["nc"] = _build_program()
    return _prog_cache["nc"]


def _shard(fix, move, Af, Am):
    in_maps = []
    for core in range(NCORES):
        b, dq = core // 4, core % 4
        lo = 20 * dq - 4
        s0, s1 = max(lo, 0), min(lo + DSLAB, D)
        slab_f = np.zeros((C, DSLAB, H, W), _bf16)
        slab_m = np.zeros((C, DSLAB, H, W), _bf16)
        slab_f[:, s0 - lo:s1 - lo] = fix[b, :, s0:s1].astype(_bf16)
        slab_m[:, s0 - lo:s1 - lo] = move[b, :, s0:s1].astype(_bf16)
        in_maps.append({"fxs": slab_f, "mvs": slab_m, "wgf": Af, "wgm": Am})
    return in_maps


def _build_weights(w_f2m, w_m2f):
    build = _build_banded_fp8 if CONV_FP8 else _build_banded_bf16
    Af = build(np.asarray(w_f2m, dtype=np.float32), 1.0)
    Am = build(np.asarray(w_m2f, dtype=np.float32), 1.0)
    return Af, Am


def kernel(fix, move, w_f2m, w_m2f, __trace=False):
    fix = np.ascontiguousarray(np.asarray(fix), dtype=np.float32)
    move = np.ascontiguousarray(np.asarray(move), dtype=np.float32)
    Af, Am = _build_weights(w_f2m, w_m2f)

    nc = _get_program()
    in_maps = _shard(fix, move, Af, Am)

    from concourse.bass_utils import run_bass_kernel_spmd

    res = run_bass_kernel_spmd(
        nc, in_maps, core_ids=list(range(NCORES)), trace=__trace
    )
    _prog_cache["last_results"] = res

    fix_out = np.empty((B, C, D, H, W), np.float32)
    move_out = np.empty((B, C, D, H, W), np.float32)
    for core in range(NCORES):
        b, dq = core // 4, core % 4
        fix_out[b, :, 20 * dq:20 * dq + 20] = res.results[core]["fo"].astype(np.float32)
        move_out[b, :, 20 * dq:20 * dq + 20] = res.results[core]["mo"].astype(np.float32)
    return fix_out, move_out


# revision 27
# speedup vs baseline: 1.1997x; 1.0284x over previous
"""Trainium2 Bass kernel for nn_CSABlock (dual spatial-attention gating).

Reference computation:
    sa_x  = sigmoid(conv3d(concat[max_c(x), mean_c(x)], w, k=7, pad=3))
    fix_out  = move * sa_fix + fix
    move_out = fix * sa_move + move

Sharding: 8 cores = (batch 2) x (D quarters of 20 planes); bf16 28-plane
slabs with 4-plane halo/pad. fp8 DoubleRow conv (stats packed into the
256-deep contraction), fp16 channel trees on DVE, DRAM bounce staging.

Queue discipline (all DMA-issue queues are in-order; a dependency-waiting
issue stalls everything behind it on that queue):
  sync   - input loads first (never wait), then late gateT reloads
  scalar - PS scale, pooled bounce writes, sigmoids, gate writes, stores
           (emission interleaved to match readiness order)
  gpsimd - P reloads (wait on bounce writes; engine otherwise idle)
"""

import sys

import numpy as np

for _p in ("/opt/trn_rl_repo",):
    if _p not in sys.path:
        sys.path.insert(0, _p)

import ml_dtypes  # noqa: E402

B, C, D, H, W = 2, 16, 80, 96, 96
KK = 7
DSLAB = 28          # padded per-core D planes (4 + 20 + 4)
OUTD = 20           # output planes per core
G = 4               # D planes per chunk / elementwise group
NCHUNK = DSLAB // G  # 7
NG = OUTD // G       # 5 elementwise groups
GC = 5               # conv-group D planes (free 5*102-6 = 504 <= 512)
NCG = OUTD // GC     # 4 conv groups
HG, HPW = 32, 3      # h = hg*3 + hp
WPAD = 102
NCORES = 8

CONV_FP8 = True

_prog_cache: dict = {}

_bf16 = ml_dtypes.bfloat16
_f8 = ml_dtypes.float8_e4m3


def _build_banded_fp8(w: np.ndarray, mean_scale: float) -> np.ndarray:
    """w: [1,2,7,7,7] f32 -> lhsT [hin_pad 128, tap 49, stat 2, hout 96] fp8."""
    A = np.zeros((128, KK * KK, 2, 96), np.float32)
    hh = np.arange(96)
    for s in range(2):
        scale = 1.0 if s == 0 else mean_scale
        for kd in range(KK):
            for kw in range(KK):
                tap = kd * KK + kw
                for kh in range(KK):
                    A[hh + kh, tap, s, hh] = w[0, s, kd, kh, kw] * scale
    return A.astype(_f8)


def _build_banded_bf16(w: np.ndarray, mean_scale: float) -> np.ndarray:
    """w: [1,2,7,7,7] f32 -> lhsT [hin_pad 128, tap 98, hout 96] bf16."""
    A = np.zeros((128, 2 * KK * KK, 96), np.float32)
    hh = np.arange(96)
    for s in range(2):
        scale = 1.0 if s == 0 else mean_scale
        for kd in range(KK):
            for kw in range(KK):
                tap = (s * KK + kd) * KK + kw
                for kh in range(KK):
                    A[hh + kh, tap, hh] = w[0, s, kd, kh, kw] * scale
    return A.astype(_bf16)


def _build_program():
    import concourse.bass as bass  # noqa: F401
    import concourse.bacc as bacc
    import concourse.tile as tile
    from concourse import mybir
    from contextlib import ExitStack

    f32 = mybir.dt.float32
    bf16 = mybir.dt.bfloat16
    f16 = mybir.dt.float16
    f8 = mybir.dt.float8e4
    pdt = f8 if CONV_FP8 else bf16
    TAPS = KK * KK if CONV_FP8 else 2 * KK * KK

    nc = bacc.Bacc("TRN2")
    fxs = nc.dram_tensor("fxs", [C, DSLAB, H, W], bf16, kind="ExternalInput")
    mvs = nc.dram_tensor("mvs", [C, DSLAB, H, W], bf16, kind="ExternalInput")
    if CONV_FP8:
        wgf = nc.dram_tensor("wgf", [128, TAPS, 2, 96], f8, kind="ExternalInput")
        wgm = nc.dram_tensor("wgm", [128, TAPS, 2, 96], f8, kind="ExternalInput")
    else:
        wgf = nc.dram_tensor("wgf", [128, TAPS, 96], bf16, kind="ExternalInput")
        wgm = nc.dram_tensor("wgm", [128, TAPS, 96], bf16, kind="ExternalInput")
    fo = nc.dram_tensor("fo", [C, OUTD, H, W], bf16, kind="ExternalOutput")
    mo = nc.dram_tensor("mo", [C, OUTD, H, W], bf16, kind="ExternalOutput")

    with tile.TileContext(nc) as tc, ExitStack() as ctx:
        singles = ctx.enter_context(tc.tile_pool(name="singles", bufs=1))
        # chunks 1-5 feed elementwise late -> own slots so loads never wait;
        # chunks 0/6 are halo-only with disjoint lifetimes (1 shared slot)
        lp = ctx.enter_context(tc.tile_pool(name="lp", bufs=5))
        lphalo = ctx.enter_context(tc.tile_pool(name="lph", bufs=1))
        trpool = ctx.enter_context(tc.tile_pool(name="tr", bufs=1))
        pspool = ctx.enter_context(tc.tile_pool(name="ps", bufs=2))
        tpool = ctx.enter_context(tc.tile_pool(name="tmp", bufs=2))
        gpool = ctx.enter_context(tc.tile_pool(name="gate", bufs=2))
        gtpool = ctx.enter_context(tc.tile_pool(name="gateT", bufs=3))
        psum = ctx.enter_context(tc.tile_pool(name="psum", bufs=4, space="PSUM"))
        dram = ctx.enter_context(tc.tile_pool(name="dram", bufs=1, space="DRAM"))

        WGF = singles.tile(list(wgf.shape), pdt)
        WGM = singles.tile(list(wgm.shape), pdt)
        nc.scalar.dma_start(out=WGF[:], in_=wgf[:])
        nc.scalar.dma_start(out=WGM[:], in_=wgm[:])

        # Persistent pooled tensors [hin_pad, stat, dp*wp]; (d,w) flattened
        # so conv rhs slices are single contiguous runs
        PF = singles.tile([128, 2, DSLAB * WPAD], pdt)
        PM = singles.tile([128, 2, DSLAB * WPAD], pdt)
        nc.gpsimd.memset(PF[:], 0.0)
        nc.gpsimd.memset(PM[:], 0.0)

        pooled_d = [
            dram.tile([2, 2, G, H, W], pdt, name=f"pooled_d{i}")
            for i in range(NCHUNK)
        ]
        gates_d = {
            (t, g): dram.tile([GC, H, W], bf16, name=f"gates_d{t}_{g}")
            for t in range(2) for g in range(NCG)
        }

        ltiles: dict = {}

        def load_and_pool(ic: int):
            i0 = G * ic
            pool_ = lp if 1 <= ic <= 5 else lphalo
            L = pool_.tile([128, 2, C, HPW * W], bf16, tag="L")
            # slab plane 27 is never read downstream: skip its load (P's
            # memset supplies zeros there; engine APs need base partition 0)
            dhi = 3 if ic == NCHUNK - 1 else G
            PL = slice(0, 32 * dhi)
            for t, dram_in in ((0, fxs), (1, mvs)):
                src = dram_in[:, i0:i0 + dhi, :, :].rearrange(
                    "c d (hg hp) w -> (d hg) c (hp w)", hg=HG, hp=HPW
                )
                nc.sync.dma_start(out=L[PL, t], in_=src)
            ltiles[ic] = L

            # channel trees over both tensors at once, fp16, all on DVE
            # (GpSimd tensor ops would hold the shared SBUF port and stall
            # DVE perf-mode ops). High priority: in the in-order DVE stream
            # trees must never queue behind elementwise - they gate convs.
            TR = trpool.tile([128, 2, 2, C // 2, HPW * W], f16, tag="TR")
            PS = pspool.tile([128, 2, 2, HPW * W], pdt, tag="PS")
            TRmax, TRsum = TR[PL, :, 0], TR[PL, :, 1]
            Lv = L[PL]
            with tc.high_priority(offset=4000):
                nc.vector.tensor_max(TRmax[:, :, :, :], Lv[:, :, 0:8, :], Lv[:, :, 8:16, :])
                nc.vector.tensor_add(TRsum[:, :, :, :], Lv[:, :, 0:8, :], Lv[:, :, 8:16, :])
                nc.vector.tensor_max(TRmax[:, :, 0:4], TRmax[:, :, 0:4], TRmax[:, :, 4:8])
                nc.vector.tensor_add(TRsum[:, :, 0:4], TRsum[:, :, 0:4], TRsum[:, :, 4:8])
                nc.vector.tensor_max(TRmax[:, :, 0:2], TRmax[:, :, 0:2], TRmax[:, :, 2:4])
                nc.vector.tensor_add(TRsum[:, :, 0:2], TRsum[:, :, 0:2], TRsum[:, :, 2:4])
                nc.vector.tensor_max(PS[PL, :, 0], TRmax[:, :, 0], TRmax[:, :, 1])
                nc.vector.tensor_add(TRsum[:, :, 0], TRsum[:, :, 0], TRsum[:, :, 1])
            # mean = sum/16 on ScalarE (1/16 in fp8 weights would hit e4m3
            # subnormals)
            nc.scalar.mul(PS[PL, :, 1], TRsum[:, :, 0], 1.0 / C)

            # bounce out: PS [(d hg), t, s, (hp w)] -> DRAM [t, s, d, h, w]
            nc.scalar.dma_start(
                out=pooled_d[ic][:, :, 0:dhi].rearrange(
                    "t s d (hg hp) w -> (d hg) t s (hp w)", hg=HG, hp=HPW
                ),
                in_=PS[PL],
            )
            # reload into conv layout, gpsimd queue (waits on the bounce
            # write; on sync it would stall later input loads)
            for t, P in ((0, PF), (1, PM)):
                Pv = P[3:99, :, :].rearrange("p s (d w) -> p s d w", w=WPAD)
                for s in range(2):
                    nc.gpsimd.dma_start(
                        out=Pv[:, s, i0:i0 + dhi, 3:3 + W],
                        in_=pooled_d[ic][t, s, 0:dhi].rearrange("d h w -> h d w"),
                    )

        NFREE = GC * WPAD - (WPAD - W)  # 504: contiguous (d,w) run per tap

        def conv_group(t: int, g: int):
            P = (PF, PM)[t]
            WG = (WGF, WGM)[t]
            o0 = GC * g
            acc = psum.tile([96, NFREE], f32, tag="acc")
            if CONV_FP8:
                for kd in range(KK):
                    for kw in range(KK):
                        tap = kd * KK + kw
                        off = (o0 + 1 + kd) * WPAD + kw
                        nc.tensor.matmul(
                            acc[:],
                            WG[:, tap],
                            P[:, :, off:off + NFREE],
                            start=(tap == 0),
                            stop=(tap == TAPS - 1),
                            perf_mode=mybir.MatmulPerfMode.DoubleRow,
                        )
            else:
                for s in range(2):
                    for kd in range(KK):
                        for kw in range(KK):
                            tap = (s * KK + kd) * KK + kw
                            off = (o0 + 1 + kd) * WPAD + kw
                            nc.tensor.matmul(
                                acc[:],
                                WG[:, tap],
                                P[:, s, off:off + NFREE],
                                start=(tap == 0),
                                stop=(tap == TAPS - 1),
                            )
            gate = gpool.tile([96, GC * WPAD], bf16, tag="gate")
            nc.scalar.activation(
                out=gate[:, 0:NFREE], in_=acc[:],
                func=mybir.ActivationFunctionType.Sigmoid,
            )
            gv = gate[:].rearrange("p (d w) -> p d w", w=WPAD)
            nc.scalar.dma_start(
                out=gates_d[(t, g)][:].rearrange("d h w -> h d w"),
                in_=gv[:, :, 0:W],
            )

        def elementwise(g: int):
            L = ltiles[g + 1]
            Ts = []
            p0 = G * g
            for t in range(2):
                gateT = gtpool.tile([128, HPW * W], bf16, tag="gT")
                # gather 4 gate planes from the (up to two) 5-plane conv
                # group tiles; sync queue is pure loads before these, all
                # already issued
                p = p0
                while p < p0 + G:
                    cg, lo = p // GC, p % GC
                    hi = min(GC, lo + (p0 + G - p))
                    nc.sync.dma_start(
                        out=gateT[32 * (p - p0):32 * (p - p0 + hi - lo)],
                        in_=gates_d[(t, cg)][lo:hi].rearrange(
                            "d (hg hp) w -> (d hg) (hp w)", hg=HG, hp=HPW
                        ),
                    )
                    p += hi - lo
                gb = gateT[:].unsqueeze(1).broadcast_to((128, C, HPW * W))
                T = tpool.tile([128, C, HPW * W], bf16, tag="T")
                # t=0: fix_out = move*gf + fix ; t=1: move_out = fix*gm + move
                nc.vector.tensor_mul(T[:], L[:, 1 - t], gb)
                nc.vector.tensor_add(T[:], T[:], L[:, t])
                Ts.append(T)
            for t, dram_out in ((0, fo), (1, mo)):
                dst = dram_out[:, p0:p0 + G, :, :].rearrange(
                    "c d (hg hp) w -> (d hg) c (hp w)", hg=HG, hp=HPW
                )
                nc.scalar.dma_start(out=dst, in_=Ts[t][:])

        # Pipeline: all loads emitted first (sync queue = loads only, and
        # lp slots guarantee no allocation wait); conv group g needs pooled
        # chunks <= [2,4,5,6][g]; elementwise g needs gates covering planes
        # 4g..4g+3 and raw chunk g+1.
        for ic in range(NCHUNK):
            load_and_pool(ic)
        conv_group(0, 0)
        conv_group(1, 0)
        conv_group(0, 1)
        conv_group(1, 1)
        elementwise(0)
        elementwise(1)
        conv_group(0, 2)
        conv_group(1, 2)
        elementwise(2)
        conv_group(0, 3)
        conv_group(1, 3)
        elementwise(3)
        elementwise(4)

    nc.compile()
    return nc


def _get_program():
    if "nc" not in _prog_cache:
        _prog_cache["nc"] = _build_program()
    return _prog_cache["nc"]


def _shard(fix, move, Af, Am):
    in_maps = []
    for core in range(NCORES):
        b, dq = core // 4, core % 4
        lo = 20 * dq - 4
        s0, s1 = max(lo, 0), min(lo + DSLAB, D)
        slab_f = np.zeros((C, DSLAB, H, W), _bf16)
        slab_m = np.zeros((C, DSLAB, H, W), _bf16)
        slab_f[:, s0 - lo:s1 - lo] = fix[b, :, s0:s1].astype(_bf16)
        slab_m[:, s0 - lo:s1 - lo] = move[b, :, s0:s1].astype(_bf16)
        in_maps.append({"fxs": slab_f, "mvs": slab_m, "wgf": Af, "wgm": Am})
    return in_maps


def _build_weights(w_f2m, w_m2f):
    build = _build_banded_fp8 if CONV_FP8 else _build_banded_bf16
    Af = build(np.asarray(w_f2m, dtype=np.float32), 1.0)
    Am = build(np.asarray(w_m2f, dtype=np.float32), 1.0)
    return Af, Am


def kernel(fix, move, w_f2m, w_m2f, __trace=False):
    fix = np.ascontiguousarray(np.asarray(fix), dtype=np.float32)
    move = np.ascontiguousarray(np.asarray(move), dtype=np.float32)
    Af, Am = _build_weights(w_f2m, w_m2f)

    nc = _get_program()
    in_maps = _shard(fix, move, Af, Am)

    from concourse.bass_utils import run_bass_kernel_spmd

    res = run_bass_kernel_spmd(
        nc, in_maps, core_ids=list(range(NCORES)), trace=__trace
    )
    _prog_cache["last_results"] = res

    fix_out = np.empty((B, C, D, H, W), np.float32)
    move_out = np.empty((B, C, D, H, W), np.float32)
    for core in range(NCORES):
        b, dq = core // 4, core % 4
        fix_out[b, :, 20 * dq:20 * dq + 20] = res.results[core]["fo"].astype(np.float32)
        move_out[b, :, 20 * dq:20 * dq + 20] = res.results[core]["mo"].astype(np.float32)
    return fix_out, move_out
